# revision 10
# baseline (speedup 1.0000x reference)
"""Trainium2 Bass kernel for nn_Block_84679575208053 (moe_routing block).

Self-contained: hardcodes B=4, T=1024, C=1024, H=16, E=8, BOT=64, TOP_K=2.

Sharding (8 cores): core c -> batch b=c//2, token half hf=c%2, and 8 of 16
attention heads (heads [hf*8, hf*8+8)).  Attention is head-parallel within a
batch pair; output-projection partials are pair-ReduceScattered so each core
ends with its 512-token half of the post-attention residual.  Routing (top-2
of 8 experts, batch 0 only) is computed on cores 0/1 (other cores get a
zeroed router) and broadcast with an even/odd-group AllReduce.  Adapter
experts (dense equivalence) and the MLP run token-parallel per core.

Heavy matmuls run in float32r (full PE rate); the router logits matmul runs
in plain float32 to keep top-2 selection faithful.
"""
import numpy as np
import concourse.mybir as mybir
from concourse import bacc, tile
from concourse.bass_utils import run_bass_kernel_spmd
from concourse.masks import make_identity

N_CORES = 8
B, T, C, H, E, BOT = 4, 1024, 1024, 16, 8, 64
P = 128
TH = T // 2          # tokens per core
NH = H // 2          # heads per core
D = C // H           # 64
DC = NH * D          # 512
HID = 4 * C          # 4096
KC = C // P          # 8
NT = T // P          # 8
NTH = TH // P        # 4
f32 = mybir.dt.float32
f32r = mybir.dt.float32r
ACT = mybir.ActivationFunctionType
ALU = mybir.AluOpType
AX = mybir.AxisListType

_CACHED_NC = None


def _bcast_row(nc, ps_pool, sb_pool, row_ap, n, name):
    """Broadcast a [1, n] f32 SBUF row to a [128, n] f32 SBUF tile via PE."""
    ones1 = sb_pool.tile([1, P], f32, name=f"{name}_ones")
    nc.vector.memset(ones1[:, :], 1.0)
    bp = ps_pool.tile([P, n], f32, name=f"{name}_bp", tag="bcast")
    for lo in range(0, n, 512):
        hi = min(n, lo + 512)
        nc.tensor.matmul(bp[:, lo:hi], ones1[:, :], row_ap[:, lo:hi],
                         start=True, stop=True)
    out = sb_pool.tile([P, n], f32, name=f"{name}_b")
    nc.scalar.activation(out[:, :], bp[:, :], ACT.Copy)
    return out


def build_nc():
    nc = bacc.Bacc(num_devices=N_CORES)

    XB = nc.declare_dram_parameter("xb", [T, C], f32, isOutput=False)
    XH = nc.declare_dram_parameter("xh", [TH, C], f32, isOutput=False)
    WQ = nc.declare_dram_parameter("wq", [C, DC], f32r, isOutput=False)
    WK = nc.declare_dram_parameter("wk", [C, DC], f32r, isOutput=False)
    WV = nc.declare_dram_parameter("wv", [C, DC], f32r, isOutput=False)
    BQV = nc.declare_dram_parameter("bqv", [3, DC], f32, isOutput=False)
    WO = nc.declare_dram_parameter("wo", [DC, C], f32r, isOutput=False)
    BO = nc.declare_dram_parameter("bo", [C], f32, isOutput=False)
    LN1G = nc.declare_dram_parameter("ln1g", [C], f32, isOutput=False)
    LN1B = nc.declare_dram_parameter("ln1b", [C], f32, isOutput=False)
    LN2G = nc.declare_dram_parameter("ln2g", [C], f32, isOutput=False)
    LN2B = nc.declare_dram_parameter("ln2b", [C], f32, isOutput=False)
    RT = nc.declare_dram_parameter("router8", [C, E], f32, isOutput=False)
    MSK = nc.declare_dram_parameter("mask01", [P, 1], f32, isOutput=False)
    ADW = nc.declare_dram_parameter("adw", [C, E * BOT], f32r, isOutput=False)
    ADB = nc.declare_dram_parameter("adb", [E * BOT], f32, isOutput=False)
    AUW = nc.declare_dram_parameter("auw", [E * BOT, C], f32r, isOutput=False)
    AUB = nc.declare_dram_parameter("aub", [E, C], f32r, isOutput=False)
    W1 = nc.declare_dram_parameter("w1", [C, HID], f32r, isOutput=False)
    B1 = nc.declare_dram_parameter("b1", [HID], f32, isOutput=False)
    W2 = nc.declare_dram_parameter("w2", [HID, C], f32r, isOutput=False)
    B2 = nc.declare_dram_parameter("b2", [C], f32, isOutput=False)
    OUT = nc.declare_dram_parameter("out", [TH, C], f32, isOutput=True)

    with tile.TileContext(nc) as tc:
        # ---------- persistent constants ----------
        cst_cm = tc.tile_pool(name="cst", bufs=1)
        cst = cst_cm.__enter__()
        with tc.tile_pool(name="cps", bufs=1, space="PSUM") as cps:
            ident = cst.tile([P, P], f32)
            make_identity(nc, ident[:, :])
            ones64 = cst.tile([D + 1, D], f32)
            nc.vector.memset(ones64[D:D + 1, :], 1.0)
            ones1f = cst.tile([1, P], f32)
            nc.vector.memset(ones1f[:, :], 1.0)
            ones1r = cst.tile([1, P], f32r)
            nc.vector.tensor_copy(ones1r[:, :], ones1f[:, :])
            onesPr = cst.tile([P, 1], f32r)
            onesPf = cst.tile([P, 1], f32)
            nc.vector.memset(onesPf[:, :], 1.0)
            nc.vector.tensor_copy(onesPr[:, :], onesPf[:, :])
            eps = cst.tile([P, 1], f32)
            nc.vector.memset(eps[:, :], 1e-5)
            mask01 = cst.tile([P, 1], f32)
            nc.sync.dma_start(out=mask01[:, :], in_=MSK[:, :])

            ln1g_r = cst.tile([1, C], f32)
            nc.sync.dma_start(out=ln1g_r[:, :], in_=LN1G[None, :])
            ln1b_r = cst.tile([1, C], f32)
            nc.sync.dma_start(out=ln1b_r[:, :], in_=LN1B[None, :])
            bo_r = cst.tile([1, C], f32)
            nc.sync.dma_start(out=bo_r[:, :], in_=BO[None, :])
            b2_r = cst.tile([1, C], f32)
            nc.sync.dma_start(out=b2_r[:, :], in_=B2[None, :])
            bv_r = cst.tile([1, DC], f32)
            nc.sync.dma_start(out=bv_r[:, :], in_=BQV[2:3, :])
            g1b = _bcast_row(nc, cps, cst, ln1g_r, C, "g1")
            b1b = _bcast_row(nc, cps, cst, ln1b_r, C, "b1l")
            bob = _bcast_row(nc, cps, cst, bo_r, C, "bo")
            b2b = _bcast_row(nc, cps, cst, b2_r, C, "b2")
            bvb = _bcast_row(nc, cps, cst, bv_r, DC, "bv")

        bq_t = cst.tile([P, 4], f32)
        nc.sync.dma_start(out=bq_t[:, :],
                          in_=BQV[0, :].rearrange("(m p) -> p m", p=P))
        bk_t = cst.tile([P, 4], f32)
        nc.sync.dma_start(out=bk_t[:, :],
                          in_=BQV[1, :].rearrange("(m p) -> p m", p=P))
        ln2g_t = cst.tile([P, KC], f32)
        nc.sync.dma_start(out=ln2g_t[:, :],
                          in_=LN2G[:].rearrange("(k p) -> p k", p=P))
        ln2b_t = cst.tile([P, KC], f32)
        nc.sync.dma_start(out=ln2b_t[:, :],
                          in_=LN2B[:].rearrange("(k p) -> p k", p=P))
        adb_t = cst.tile([P, 4], f32)
        nc.sync.dma_start(out=adb_t[:, :],
                          in_=ADB[:].rearrange("(m p) -> p m", p=P))
        b1_t = cst.tile([P, HID // P], f32)
        nc.sync.dma_start(out=b1_t[:, :],
                          in_=B1[:].rearrange("(m p) -> p m", p=P))
        # expert selector: sel4[e, f] = 1 iff f//64 == e  (f = mb*128 + p)
        it8 = cst.tile([E, E, D], f32)
        nc.gpsimd.iota(it8[:, :, :], pattern=[[1, E], [0, D]], base=0,
                       channel_multiplier=-1,
                       allow_small_or_imprecise_dtypes=True)
        sel4_f = cst.tile([E, 4, P], f32)
        nc.vector.tensor_scalar(
            out=sel4_f.rearrange("e a b -> e (a b)"),
            in0=it8.rearrange("e a b -> e (a b)"),
            scalar1=0.0, scalar2=None, op0=ALU.is_equal)
        sel4 = cst.tile([E, 4, P], f32r)
        nc.vector.tensor_copy(sel4[:, :, :], sel4_f[:, :, :])

        dram_cm = tc.tile_pool(name="dram", bufs=1, space="DRAM")
        dram = dram_cm.__enter__()
        rs_in = dram.tile([T, C], f32)
        rs_out = dram.tile([TH, C], f32)
        g_in = dram.tile([TH, E], f32)
        g_out = dram.tile([TH, E], f32)

        # ---------- long-lived tensors (LIFO stack bottom) ----------
        # perm pool: slots are tag-shared across disjoint lifetimes
        perm_cm = tc.tile_pool(name="perm", bufs=1)
        perm = perm_cm.__enter__()
        hT_a = perm.tile([P, 4, T], f32r, tag="pA")   # h^T chunks 0-3
        hT_b = perm.tile([P, 4, T], f32r, tag="pB")   # h^T chunks 4-7
        vona = perm.tile([P, NT, NH, D + 1], f32r, tag="pC")
        qt = perm.tile([P, 4, T], f32r, tag="pD")     # Q^T [d, t]
        kt = perm.tile([P, 4, T], f32r, tag="pE")
        moe_cm = tc.tile_pool(name="moe", bufs=1)
        moe = moe_cm.__enter__()
        gatesT = moe.tile([E, TH], f32r)
        att_cm = tc.tile_pool(name="att", bufs=1)
        att = att_cm.__enter__()
        yt2 = [att.tile([P, T], f32r, name=f"yt2_{i}") for i in range(4)]

        def hTc(k):
            return hT_a[:, k, :] if k < 4 else hT_b[:, k - 4, :]

        # ---------- phase A: LN1 + transpose -> hT ----------
        with tc.tile_pool(name="pa", bufs=2) as pa, \
             tc.tile_pool(name="pap", bufs=4, space="PSUM") as pap:
            for tt in range(NT):
                xr = pa.tile([P, C], f32, tag="xr")
                nc.sync.dma_start(out=xr[:, :], in_=XB[tt * P:(tt + 1) * P, :])
                s1 = pa.tile([P, 1], f32, tag="s1")
                nc.vector.reduce_sum(out=s1[:, :], in_=xr[:, :], axis=AX.X)
                mu = pa.tile([P, 1], f32, tag="mu")
                nc.vector.tensor_scalar_mul(mu[:, :], s1[:, :], 1.0 / C)
                sq = pa.tile([P, C], f32, tag="sq")
                ss = pa.tile([P, 1], f32, tag="ss")
                nc.scalar.activation(sq[:, :], xr[:, :], ACT.Square,
                                     accum_out=ss[:, :])
                mu2 = pa.tile([P, 1], f32, tag="mu2")
                nc.vector.tensor_mul(mu2[:, :], mu[:, :], mu[:, :])
                var = pa.tile([P, 1], f32, tag="var")
                nc.vector.scalar_tensor_tensor(
                    out=var[:, :], in0=ss[:, :], scalar=1.0 / C,
                    in1=mu2[:, :], op0=ALU.mult, op1=ALU.subtract)
                sd = pa.tile([P, 1], f32, tag="sd")
                nc.scalar.activation(sd[:, :], var[:, :], ACT.Sqrt,
                                     bias=eps[:, :])
                inv = pa.tile([P, 1], f32, tag="inv")
                nc.vector.reciprocal(inv[:, :], sd[:, :])
                hrow = pa.tile([P, C], f32, tag="hrow")
                nc.vector.tensor_scalar(
                    out=hrow[:, :], in0=xr[:, :], scalar1=mu[:, :],
                    scalar2=inv[:, :], op0=ALU.subtract, op1=ALU.mult)
                nc.vector.tensor_mul(hrow[:, :], hrow[:, :], g1b[:, :])
                nc.vector.tensor_add(hrow[:, :], hrow[:, :], b1b[:, :])
                for cc in range(KC):
                    trp = pap.tile([P, P], f32, tag="trp")
                    nc.tensor.transpose(trp[:, :],
                                        hrow[:, cc * P:(cc + 1) * P],
                                        ident[:, :])
                    nc.scalar.activation(
                        hTc(cc)[:, tt * P:(tt + 1) * P], trp[:, :], ACT.Copy)

        # ---------- phase B: QKV projections ----------
        with tc.tile_pool(name="pb", bufs=1) as pb, \
             tc.tile_pool(name="pbp", bufs=2, space="PSUM") as pbp:
            wv_s = pb.tile([P, KC, DC], f32r, tag="wv")
            nc.sync.dma_start(out=wv_s[:, :, :],
                              in_=WV.rearrange("(k p) m -> p k m", p=P))
            for (W_d, dest, b_t) in ((WQ, qt, bq_t), (WK, kt, bk_t)):
                for dm in range(4):
                    wqk = pb.tile([P, KC, P], f32r, tag="wqk", bufs=2)
                    nc.sync.dma_start(
                        out=wqk[:, :, :],
                        in_=W_d.rearrange("(k p) m -> p k m", p=P)[
                            :, :, dm * P:(dm + 1) * P])
                    for th2 in range(2):
                        lo = th2 * 512
                        pp = pbp.tile([P, 512], f32, tag="pqk")
                        for k in range(KC):
                            nc.tensor.matmul(
                                pp[:, :], wqk[:, k, :],
                                hTc(k)[:, lo:lo + 512], start=(k == 0),
                                stop=(k == KC - 1))
                        nc.scalar.activation(dest[:, dm, lo:lo + 512],
                                             pp[:, :], ACT.Identity,
                                             bias=b_t[:, dm:dm + 1])
            onesf = pb.tile([P, NH], f32, tag="onesf")
            nc.vector.memset(onesf[:, :], 1.0)
            for j in range(NT):
                pv = pbp.tile([P, DC], f32, tag="pv")
                for k in range(KC):
                    nc.tensor.matmul(pv[:, :], hTc(k)[:, j * P:(j + 1) * P],
                                     wv_s[:, k, :], start=(k == 0),
                                     stop=(k == KC - 1))
                nc.vector.tensor_tensor(
                    out=vona[:, j, :, 0:D],
                    in0=pv.rearrange("p (h d) -> p h d", h=NH),
                    in1=bvb.rearrange("p (h d) -> p h d", h=NH), op=ALU.add)
                nc.vector.tensor_copy(
                    vona[:, j, :, D:D + 1].squeeze(-1), onesf[:, :])

        # ---------- phase C: attention per head ----------
        with tc.tile_pool(name="pc", bufs=2) as pc, \
             tc.tile_pool(name="pcs", bufs=2, space="PSUM") as pcs, \
             tc.tile_pool(name="pcy", bufs=1, space="PSUM") as pcy, \
             tc.tile_pool(name="pcr", bufs=1, space="PSUM") as pcr:
            for h in range(NH):
                ht, hh = h // 2, (h % 2) * D
                yp = pcy.tile([D + 1, T], f32, tag="yp")
                for j in range(NT):
                    t0 = j * P
                    if t0 < 512:
                        spans = [(t0, 512), (512, T)]
                    else:
                        spans = [(t0, T)]
                    sp = pcs.tile([P, T], f32, tag="sp")
                    es = pc.tile([P, T], f32r, tag="es")
                    for (lo, hi) in spans:
                        nc.tensor.matmul(
                            sp[:, lo:hi],
                            kt[hh:hh + D, ht, t0:t0 + P],
                            qt[hh:hh + D, ht, lo:hi],
                            start=True, stop=True)
                        nc.scalar.activation(es[:, lo:hi], sp[:, lo:hi],
                                             ACT.Exp, scale=0.125)
                    nc.gpsimd.affine_select(
                        out=es[:, t0:t0 + P], in_=es[:, t0:t0 + P],
                        compare_op=ALU.is_ge, fill=0.0,
                        base=0, pattern=[[1, P]], channel_multiplier=-1)
                    for (lo, hi) in spans:
                        nc.tensor.matmul(yp[:, lo:hi], vona[:, j, h, :],
                                         es[:, lo:hi], start=(j == 0),
                                         stop=(j == NT - 1))
                rec = pc.tile([D + 1, T], f32, tag="rec")
                nc.vector.reciprocal(rec[D:D + 1, :], yp[D:D + 1, :])
                rb = pcr.tile([D, T], f32, tag="rb")
                for lo in range(0, T, 512):
                    nc.tensor.matmul(rb[:, lo:lo + 512],
                                     ones64[D:D + 1, :],
                                     rec[D:D + 1, lo:lo + 512],
                                     start=True, stop=True)
                rbs = pc.tile([D, T], f32, tag="rbs")
                nc.scalar.activation(rbs[:, :], rb[:, :], ACT.Copy)
                if h % 2 == 0:
                    nc.vector.tensor_tensor(out=yt2[ht][0:D, :],
                                            in0=yp[:D, :],
                                            in1=rbs[:, :], op=ALU.mult)
                else:
                    ytmp = pc.tile([D, T], f32r, tag="ytmp")
                    nc.vector.tensor_tensor(out=ytmp[:, :], in0=yp[:D, :],
                                            in1=rbs[:, :], op=ALU.mult)
                    nc.sync.dma_start(out=yt2[ht][D:P, :], in_=ytmp[:, :])

        # ---------- phase D: Wo partial + pair ReduceScatter ----------
        xn = perm.tile([P, NTH, C], f32, tag="pA")     # x_new (t-major)
        xnT = perm.tile([P, KC, TH], f32r, tag="pB")   # x_new^T
        xnT32 = perm.tile([P, KC, TH], f32, tag="pD")  # f32 copy for router
        with tc.tile_pool(name="pdw", bufs=1) as pdw, \
             tc.tile_pool(name="pd", bufs=2) as pd, \
             tc.tile_pool(name="pdp", bufs=2, space="PSUM") as pdp:
            wo_s = pdw.tile([P, 4, C], f32r, tag="wo")
            nc.sync.dma_start(out=wo_s[:, :, :],
                              in_=WO.rearrange("(k p) m -> p k m", p=P))
            for tt in range(NT):
                par = pd.tile([P, C], f32, tag="par")
                for ch in range(2):
                    lo = ch * 512
                    pw = pdp.tile([P, 512], f32, tag="pw")
                    for k in range(4):
                        nc.tensor.matmul(
                            pw[:, :], yt2[k][:, tt * P:(tt + 1) * P],
                            wo_s[:, k, lo:lo + 512],
                            start=(k == 0), stop=(k == 3))
                    nc.scalar.activation(par[:, lo:lo + 512], pw[:, :],
                                         ACT.Copy)
                nc.sync.dma_start(out=rs_in[tt * P:(tt + 1) * P, :],
                                  in_=par[:, :])
            nc.gpsimd.collective_compute(
                "ReduceScatter", ALU.add,
                replica_groups=[[0, 1], [2, 3], [4, 5], [6, 7]],
                ins=[rs_in[:, :]], outs=[rs_out[:, :]])
            for tt in range(NTH):
                rso = pd.tile([P, C], f32, tag="rso")
                nc.sync.dma_start(out=rso[:, :],
                                  in_=rs_out[tt * P:(tt + 1) * P, :])
                xh_t = pd.tile([P, C], f32, tag="xh")
                nc.sync.dma_start(out=xh_t[:, :],
                                  in_=XH[tt * P:(tt + 1) * P, :])
                nc.vector.tensor_add(xn[:, tt, :], rso[:, :], xh_t[:, :])
                nc.vector.tensor_add(xn[:, tt, :], xn[:, tt, :], bob[:, :])
                for cc in range(KC):
                    trp2 = pdp.tile([P, P], f32, tag="trp2")
                    nc.tensor.transpose(trp2[:, :],
                                        xn[:, tt, cc * P:(cc + 1) * P],
                                        ident[:, :])
                    nc.scalar.activation(
                        xnT[:, cc, tt * P:(tt + 1) * P], trp2[:, :], ACT.Copy)
                    nc.vector.tensor_copy(
                        xnT32[:, cc, tt * P:(tt + 1) * P], trp2[:, :])
        att_cm.__exit__(None, None, None)

        # ---------- phase E: routing ----------
        with tc.tile_pool(name="pe", bufs=2) as pe, \
             tc.tile_pool(name="pep", bufs=2, space="PSUM") as pep:
            rt_s = pe.tile([P, KC, E], f32, tag="rt", bufs=1)
            nc.sync.dma_start(out=rt_s[:, :, :],
                              in_=RT.rearrange("(k p) e -> p k e", p=P))
            for tt in range(NTH):
                lp = pep.tile([P, E], f32, tag="lp")
                for k in range(KC):
                    nc.tensor.matmul(lp[:, :],
                                     xnT32[:, k, tt * P:(tt + 1) * P],
                                     rt_s[:, k, :], start=(k == 0),
                                     stop=(k == KC - 1))
                lg = pe.tile([P, E], f32, tag="lg")
                nc.scalar.activation(lg[:, :], lp[:, :], ACT.Copy)
                m1 = pe.tile([P, 1], f32, tag="m1")
                nc.vector.reduce_max(out=m1[:, :], in_=lg[:, :], axis=AX.X)
                mk1 = pe.tile([P, E], f32, tag="mk1")
                nc.vector.tensor_scalar(out=mk1[:, :], in0=lg[:, :],
                                        scalar1=m1[:, :], scalar2=None,
                                        op0=ALU.is_equal)
                msk_l = pe.tile([P, E], f32, tag="mskl")
                nc.vector.scalar_tensor_tensor(
                    out=msk_l[:, :], in0=mk1[:, :], scalar=-1e30,
                    in1=lg[:, :], op0=ALU.mult, op1=ALU.add)
                m2 = pe.tile([P, 1], f32, tag="m2")
                nc.vector.reduce_max(out=m2[:, :], in_=msk_l[:, :], axis=AX.X)
                mk2 = pe.tile([P, E], f32, tag="mk2")
                nc.vector.tensor_scalar(out=mk2[:, :], in0=msk_l[:, :],
                                        scalar1=m2[:, :], scalar2=None,
                                        op0=ALU.is_equal)
                dd = pe.tile([P, 1], f32, tag="dd")
                nc.vector.tensor_sub(dd[:, :], m2[:, :], m1[:, :])
                ee = pe.tile([P, 1], f32, tag="ee")
                nc.scalar.activation(ee[:, :], dd[:, :], ACT.Exp)
                ep1 = pe.tile([P, 1], f32, tag="ep1")
                nc.vector.tensor_scalar_add(ep1[:, :], ee[:, :], 1.0)
                gg1 = pe.tile([P, 1], f32, tag="gg1")
                nc.vector.reciprocal(gg1[:, :], ep1[:, :])
                gg2 = pe.tile([P, 1], f32, tag="gg2")
                nc.vector.tensor_mul(gg2[:, :], ee[:, :], gg1[:, :])
                gt = pe.tile([P, E], f32, tag="gt")
                nc.vector.tensor_scalar_mul(gt[:, :], mk1[:, :], gg1[:, :])
                gt2 = pe.tile([P, E], f32, tag="gt2")
                nc.vector.tensor_scalar_mul(gt2[:, :], mk2[:, :], gg2[:, :])
                nc.vector.tensor_add(gt[:, :], gt[:, :], gt2[:, :])
                nc.vector.tensor_scalar_mul(gt[:, :], gt[:, :], mask01[:, :])
                nc.sync.dma_start(out=g_in[tt * P:(tt + 1) * P, :],
                                  in_=gt[:, :])
            nc.gpsimd.collective_compute(
                "AllReduce", ALU.add,
                replica_groups=[[0, 2, 4, 6], [1, 3, 5, 7]],
                ins=[g_in[:, :]], outs=[g_out[:, :]])
            gf = pe.tile([P, NTH, E], f32, tag="gf", bufs=1)
            nc.sync.dma_start(out=gf[:, :, :],
                              in_=g_out.rearrange("(tt p) e -> p tt e", p=P))
            for tt in range(NTH):
                gtp = pep.tile([P, P], f32, tag="gtp")
                nc.tensor.transpose(gtp[:E, :], gf[:, tt, :], ident[:, :])
                nc.scalar.activation(gatesT[:, tt * P:(tt + 1) * P],
                                     gtp[:E, :], ACT.Copy)

        # ---------- phase F: MoE adapters ----------
        ya = perm.tile([P, NTH, C], f32, tag="pE")
        with tc.tile_pool(name="pf", bufs=1) as pf, \
             tc.tile_pool(name="pfp", bufs=2, space="PSUM") as pfp:
            ad_s = pf.tile([P, KC, E * BOT], f32r, tag="ad")
            nc.sync.dma_start(out=ad_s[:, :, :],
                              in_=ADW.rearrange("(k p) m -> p k m", p=P))
            au_s = pf.tile([P, 4, C], f32r, tag="au")
            nc.sync.dma_start(out=au_s[:, :, :],
                              in_=AUW.rearrange("(k p) m -> p k m", p=P))
            aub_s = pf.tile([E, C], f32r, tag="aub")
            nc.sync.dma_start(out=aub_s[:, :], in_=AUB[:, :])
            dwn = pf.tile([P, 4, TH], f32r, tag="dwn")
            for mb in range(4):
                dp = pfp.tile([P, TH], f32, tag="dp")
                for k in range(KC):
                    nc.tensor.matmul(dp[:, :],
                                     ad_s[:, k, mb * P:(mb + 1) * P],
                                     xnT[:, k, :], start=(k == 0),
                                     stop=(k == KC - 1))
                nc.scalar.activation(dwn[:, mb, :], dp[:, :], ACT.Relu,
                                     bias=adb_t[:, mb:mb + 1])
                gb = pfp.tile([P, TH], f32, tag="gb")
                nc.tensor.matmul(gb[:, :], sel4[:, mb, :], gatesT[:, :],
                                 start=True, stop=True)
                nc.vector.tensor_tensor(out=dwn[:, mb, :], in0=dwn[:, mb, :],
                                        in1=gb[:, :], op=ALU.mult)
            for tt in range(NTH):
                up = pfp.tile([P, C], f32, tag="up")
                for ch in range(2):
                    lo = ch * 512
                    for mb in range(4):
                        nc.tensor.matmul(
                            up[:, lo:lo + 512],
                            dwn[:, mb, tt * P:(tt + 1) * P],
                            au_s[:, mb, lo:lo + 512],
                            start=(mb == 0), stop=False)
                    nc.tensor.matmul(
                        up[:, lo:lo + 512],
                        gatesT[:, tt * P:(tt + 1) * P],
                        aub_s[:, lo:lo + 512], start=False, stop=True)
                nc.scalar.activation(ya[:, tt, :], up[:, :], ACT.Copy,
                                     scale=0.1)

        # ---------- phase G: LN2 -> h2T ----------
        h2T = perm.tile([P, KC, TH], f32r, tag="pC")
        with tc.tile_pool(name="pg", bufs=1) as pg, \
             tc.tile_pool(name="pgp", bufs=1, space="PSUM") as pgp:
            sqT = pg.tile([P, KC, TH], f32r, tag="sqT")
            nc.scalar.activation(sqT[:, :, :], xnT[:, :, :], ACT.Square)
            mu_p = pgp.tile([1, TH], f32, tag="mup")
            ss_p = pgp.tile([1, TH], f32, tag="ssp")
            for k in range(KC):
                nc.tensor.matmul(mu_p[:, :], onesPr[:, :], xnT[:, k, :],
                                 start=(k == 0), stop=(k == KC - 1))
                nc.tensor.matmul(ss_p[:, :], onesPr[:, :], sqT[:, k, :],
                                 start=(k == 0), stop=(k == KC - 1))
            mu_r = pg.tile([1, TH], f32, tag="mur")
            nc.vector.tensor_scalar_mul(mu_r[:, :], mu_p[:, :], 1.0 / C)
            mu2_r = pg.tile([1, TH], f32, tag="mu2r")
            nc.vector.tensor_mul(mu2_r[:, :], mu_r[:, :], mu_r[:, :])
            var_r = pg.tile([1, TH], f32, tag="varr")
            nc.vector.scalar_tensor_tensor(
                out=var_r[:, :], in0=ss_p[:, :], scalar=1.0 / C,
                in1=mu2_r[:, :], op0=ALU.mult, op1=ALU.subtract)
            sd_r = pg.tile([1, TH], f32, tag="sdr")
            nc.scalar.activation(sd_r[:, :], var_r[:, :], ACT.Sqrt,
                                 bias=eps[0:1, :])
            inv_r = pg.tile([1, TH], f32, tag="invr")
            nc.vector.reciprocal(inv_r[:, :], sd_r[:, :])
            mu_fr = pg.tile([1, TH], f32r, tag="mufr")
            nc.vector.tensor_copy(mu_fr[:, :], mu_r[:, :])
            inv_fr = pg.tile([1, TH], f32r, tag="invfr")
            nc.vector.tensor_copy(inv_fr[:, :], inv_r[:, :])
            mu_b = pgp.tile([P, TH], f32, tag="mub")
            inv_b = pgp.tile([P, TH], f32, tag="invb")
            nc.tensor.matmul(mu_b[:, :], ones1r[:, :], mu_fr[:, :],
                             start=True, stop=True)
            nc.tensor.matmul(inv_b[:, :], ones1r[:, :], inv_fr[:, :],
                             start=True, stop=True)
            for k in range(KC):
                nc.vector.tensor_tensor(out=h2T[:, k, :], in0=xnT[:, k, :],
                                        in1=mu_b[:, :], op=ALU.subtract)
                nc.vector.tensor_tensor(out=h2T[:, k, :], in0=h2T[:, k, :],
                                        in1=inv_b[:, :], op=ALU.mult)
                nc.vector.tensor_scalar(
                    out=h2T[:, k, :], in0=h2T[:, k, :],
                    scalar1=ln2g_t[:, k:k + 1], scalar2=ln2b_t[:, k:k + 1],
                    op0=ALU.mult, op1=ALU.add)

        # ---------- phase H: MLP + final combine ----------
        GRP = 4
        NG = (HID // P) // GRP   # 8 groups of 4 hid-chunks
        with tc.tile_pool(name="ph", bufs=1) as ph, \
             tc.tile_pool(name="php", bufs=2, space="PSUM") as php, \
             tc.tile_pool(name="php2", bufs=2, space="PSUM") as php2:
            m2s = ph.tile([P, NTH, C], f32, tag="m2s", bufs=1)
            for g in range(NG):
                m1s = []
                for mi in range(GRP):
                    m = g * GRP + mi
                    w1t = ph.tile([P, KC, P], f32r, tag="w1t", bufs=2)
                    nc.sync.dma_start(
                        out=w1t[:, :, :],
                        in_=W1.rearrange("(k p) m -> p k m", p=P)[
                            :, :, m * P:(m + 1) * P])
                    mp = php.tile([P, TH], f32, tag="mp")
                    for k in range(KC):
                        nc.tensor.matmul(mp[:, :], w1t[:, k, :],
                                         h2T[:, k, :], start=(k == 0),
                                         stop=(k == KC - 1))
                    m1 = ph.tile([P, TH], f32r, tag="m1", bufs=GRP + 2)
                    nc.scalar.activation(m1[:, :], mp[:, :], ACT.Gelu,
                                         bias=b1_t[:, m:m + 1])
                    w2t = ph.tile([P, C], f32r, tag="w2t", bufs=GRP + 2)
                    nc.sync.dma_start(out=w2t[:, :],
                                      in_=W2[m * P:(m + 1) * P, :])
                    m1s.append((m1, w2t))
                for tt in range(NTH):
                    m2p = php2.tile([P, C], f32, tag="m2p")
                    for ch in range(2):
                        lo = ch * 512
                        for mi in range(GRP):
                            m1t, w2t = m1s[mi]
                            nc.tensor.matmul(
                                m2p[:, lo:lo + 512],
                                m1t[:, tt * P:(tt + 1) * P],
                                w2t[:, lo:lo + 512],
                                start=(mi == 0), stop=(mi == GRP - 1))
                    if g == 0:
                        nc.scalar.activation(m2s[:, tt, :], m2p[:, :],
                                             ACT.Copy)
                    else:
                        nc.vector.tensor_tensor(out=m2s[:, tt, :],
                                                in0=m2s[:, tt, :],
                                                in1=m2p[:, :], op=ALU.add)
            for tt in range(NTH):
                oo = ph.tile([P, C], f32, tag="oo", bufs=2)
                nc.vector.tensor_add(oo[:, :], xn[:, tt, :], ya[:, tt, :])
                nc.vector.tensor_add(oo[:, :], oo[:, :], m2s[:, tt, :])
                nc.vector.tensor_add(oo[:, :], oo[:, :], b2b[:, :])
                nc.sync.dma_start(out=OUT[tt * P:(tt + 1) * P, :],
                                  in_=oo[:, :])

        moe_cm.__exit__(None, None, None)
        perm_cm.__exit__(None, None, None)
        dram_cm.__exit__(None, None, None)
        cst_cm.__exit__(None, None, None)
    nc.compile()
    return nc


def get_nc():
    global _CACHED_NC
    if _CACHED_NC is None:
        _CACHED_NC = build_nc()
    return _CACHED_NC


def kernel(**inputs):
    x = np.asarray(inputs["x"], np.float32)
    Wq = np.asarray(inputs["Wq"], np.float32)
    Wk = np.asarray(inputs["Wk"], np.float32)
    Wv = np.asarray(inputs["Wv"], np.float32)
    bq = np.asarray(inputs["bq"], np.float32)
    bk = np.asarray(inputs["bk"], np.float32)
    bv = np.asarray(inputs["bv"], np.float32)
    Wo = np.asarray(inputs["Wo"], np.float32)
    bo = np.asarray(inputs["bo"], np.float32)
    ln1_g = np.asarray(inputs["ln1_g"], np.float32)
    ln1_b = np.asarray(inputs["ln1_b"], np.float32)
    ln2_g = np.asarray(inputs["ln2_g"], np.float32)
    ln2_b = np.asarray(inputs["ln2_b"], np.float32)
    mlp_w1 = np.asarray(inputs["mlp_w1"], np.float32)
    mlp_b1 = np.asarray(inputs["mlp_b1"], np.float32)
    mlp_w2 = np.asarray(inputs["mlp_w2"], np.float32)
    mlp_b2 = np.asarray(inputs["mlp_b2"], np.float32)
    router = np.asarray(inputs["router"], np.float32)
    ad_w = np.asarray(inputs["ad_w"], np.float32)
    ad_b = np.asarray(inputs["ad_b"], np.float32)
    au_w = np.asarray(inputs["au_w"], np.float32)
    au_b = np.asarray(inputs["au_b"], np.float32)
    n_head = int(np.asarray(inputs["n_head"]))
    assert n_head == H

    nc = get_nc()
    adw2 = np.ascontiguousarray(ad_w.transpose(1, 0, 2).reshape(C, E * BOT))
    auw2 = np.ascontiguousarray(au_w.reshape(E * BOT, C))
    adb2 = np.ascontiguousarray(ad_b.reshape(E * BOT))
    zr = np.zeros_like(router)
    in_maps = []
    for c in range(N_CORES):
        b, hf = c // 2, c % 2
        cs = slice(hf * DC, (hf + 1) * DC)
        in_maps.append({
            "xb": np.ascontiguousarray(x[b]),
            "xh": np.ascontiguousarray(x[b, hf * TH:(hf + 1) * TH]),
            "wq": np.ascontiguousarray(Wq[:, cs]),
            "wk": np.ascontiguousarray(Wk[:, cs]),
            "wv": np.ascontiguousarray(Wv[:, cs]),
            "bqv": np.ascontiguousarray(np.stack([bq[cs], bk[cs], bv[cs]])),
            "wo": np.ascontiguousarray(Wo[cs, :]),
            "bo": bo, "ln1g": ln1_g, "ln1b": ln1_b,
            "ln2g": ln2_g, "ln2b": ln2_b,
            "router8": router if c < 2 else zr,
            "mask01": np.full((P, 1), 1.0 if c < 2 else 0.0, np.float32),
            "adw": adw2, "adb": adb2, "auw": auw2, "aub": au_b,
            "w1": mlp_w1, "b1": mlp_b1, "w2": mlp_w2, "b2": mlp_b2,
        })
    res = run_bass_kernel_spmd(nc, in_maps, list(range(N_CORES))).results
    out = np.empty((B, T, C), np.float32)
    for c in range(N_CORES):
        b, hf = c // 2, c % 2
        out[b, hf * TH:(hf + 1) * TH] = res[c]["out"]
    return out
